# revision 6
# baseline (speedup 1.0000x reference)
"""ExpSyn kernel: diagonal linear recurrence isyn_t = beta*isyn_{t-1} + x_t.

The DVE tensor_tensor_scan runs at ~2.12 ns/col regardless of dtype (serial
dependency), so a plain per-row scan is DVE-bound at ~70us/core. This kernel
uses an odd-even (Sklansky-style) decomposition of depth D=4 to convert most
of the scan into elementwise FMAs spread across THREE engines:

  pack level l:  x^(l)_i = beta^(2^(l-1)) * x^(l-1)_{2i} + x^(l-1)_{2i+1}
  scan (T/16 cols) on the deepest level only
  unpack level l: y^(l-1)_{2i} = beta^(2^(l-1)) * y^(l)_{i-1} + x^(l-1)_{2i}
                  y^(l-1)_{2i+1} = y^(l)_i   (free -- already in place)

Data is laid out host-side in a "stream" permutation (col = (t%16)*256 +
t//16) so every pack/unpack is a full-width stride-1 op. The multiply halves
of the big levels run on ACT (0.9 ns/col, per-partition scale), the level-1
add runs on GPSIMD (2.17 ns/col), the rest runs on DVE (tensor_scalar at 4x
= 0.31, tensor_tensor at 2x = 0.55, scalar_tensor_tensor at 1.1 ns/col).
Block 0 keeps the simple chunked plain-scan so the pipeline head stays short
(the tree's ACT->GPS->DVE chain has ~10us latency).

I/O is fp16 (HBM-traffic halving; scan state is fp32 internally; measured
rel err ~5e-4 vs the 2e-2 gate). Loads ride the sync ring, stores the PE
ring, ACT/GPS/DVE do compute only.
"""

import numpy as np

DT = 1e-4
B, T, N = 16, 4096, 512
NCORES = 8
BLOC = B // NCORES          # 2 batches per core
ROWS = BLOC * N             # 1024 scan rows per core
NG = N // 128               # 4 channel groups of 128
NTILES = ROWS // 128        # 8 row-blocks per core
D = 4                       # odd-even tree depth
M = 1 << D                  # 16 streams
S = T >> D                  # 256 cols per stream
NPOW = D + 1                # beta powers 2^0 .. 2^D

_cached = None


def _build():
    import concourse.bacc as bacc
    import concourse.mybir as mybir
    from concourse import tile

    nc = bacc.Bacc("TRN2", debug=False, num_devices=NCORES)
    f32 = mybir.dt.float32
    f16 = mybir.dt.float16
    mult, add = mybir.AluOpType.mult, mybir.AluOpType.add

    x = nc.dram_tensor("x", [ROWS, T], f16, kind="ExternalInput")
    beta_d = nc.dram_tensor("beta", [128, NG * NPOW], f32, kind="ExternalInput")
    y = nc.dram_tensor("y", [ROWS, T], f16, kind="ExternalOutput")

    def gv(ap, ngroups, lo, hi, step):
        """Grouped stream view: [128, ngroups, width] selecting streams."""
        return ap.rearrange("p (g s) -> p g s", g=ngroups)[:, lo:hi:step, :]

    with tile.TileContext(nc) as tc:
        with (
            tc.tile_pool(name="const", bufs=1) as cpool,
            tc.tile_pool(name="work", bufs=3) as wpool,
            tc.tile_pool(name="b0", bufs=1) as b0pool,
        ):
            bsb = cpool.tile([128, NG * NPOW], f32, name="bsb")
            nc.scalar.dma_start(out=bsb[:, :], in_=beta_d[:, :])

            def pw(g, j):            # [128,1] scalar: beta^(2^j) for group g
                return bsb[:, g * NPOW + j:g * NPOW + j + 1]

            def pwb(g, j, n):        # broadcast for the scan
                return pw(g, j).broadcast_to([128, n])

            # ---- block 0: chunked plain scan (short pipeline head) ----
            bounds = [0, 128, 512, 1536, T]
            xt0 = b0pool.tile([128, T], f16, name="xt0")
            for c in range(len(bounds) - 1):
                lo, hi = bounds[c], bounds[c + 1]
                nc.sync.dma_start(out=xt0[:, lo:hi], in_=x[0:128, lo:hi])
            for c in range(len(bounds) - 1):
                lo, hi = bounds[c], bounds[c + 1]
                init = 0.0 if c == 0 else xt0[:, lo - 1:lo]
                nc.vector.tensor_tensor_scan(
                    xt0[:, lo:hi], pwb(0, 0, hi - lo), xt0[:, lo:hi],
                    init, mult, add)
            nc.gpsimd.dma_start(out=y[0:128, :], in_=xt0[:, :])

            # ---- blocks 1..7: odd-even tree ----
            for k in range(1, NTILES):
                g = k % NG
                r0 = k * 128
                X0 = wpool.tile([128, T], f16, tag="x0", name=f"x0_{k}")
                TM1 = wpool.tile([128, T // 2], f16, tag="tm1", name=f"tm1_{k}")
                X1 = wpool.tile([128, T // 2], f16, tag="x1", name=f"x1_{k}")
                X2 = wpool.tile([128, T // 4], f16, tag="x2", name=f"x2_{k}")
                X3 = wpool.tile([128, T // 8], f16, tag="x3", name=f"x3_{k}")
                X4 = wpool.tile([128, T // 16], f16, tag="x4", name=f"x4_{k}")
                Y = wpool.tile([128, T], f16, tag="y", name=f"y_{k}")
                TE1b = wpool.tile([128, 7 * S], f16, tag="te1b", name=f"te1b_{k}")
                TE2b = wpool.tile([128, 3 * S], f16, tag="te2b", name=f"te2b_{k}")
                TE1e = wpool.tile([128, S], f16, tag="te1e", name=f"te1e_{k}")
                TE2e = wpool.tile([128, S], f16, tag="te2e", name=f"te2e_{k}")

                nc.sync.dma_start(out=X0[:, :], in_=x[r0:r0 + 128, :])

                # L1 pack: ACT mult + GPSIMD add
                nc.scalar.mul(gv(TM1[:, :], 8, 0, 8, 1),
                              gv(X0[:, :], 16, 0, 16, 2), pw(g, 0))
                nc.gpsimd.tensor_tensor(
                    out=X1[:, :].rearrange("p (g s) -> p g s", g=8),
                    in0=gv(TM1[:, :], 8, 0, 8, 1),
                    in1=gv(X0[:, :], 16, 1, 16, 2), op=add)
                # L2..L4 packs: DVE stt
                nc.vector.scalar_tensor_tensor(
                    out=X2[:, :].rearrange("p (g s) -> p g s", g=4),
                    in0=gv(X1[:, :], 8, 0, 8, 2), scalar=pw(g, 1),
                    in1=gv(X1[:, :], 8, 1, 8, 2), op0=mult, op1=add)
                nc.vector.scalar_tensor_tensor(
                    out=X3[:, :].rearrange("p (g s) -> p g s", g=2),
                    in0=gv(X2[:, :], 4, 0, 4, 2), scalar=pw(g, 2),
                    in1=gv(X2[:, :], 4, 1, 4, 2), op0=mult, op1=add)
                nc.vector.scalar_tensor_tensor(
                    out=X4[:, :], in0=X3[:, 0:S], scalar=pw(g, 3),
                    in1=X3[:, S:2 * S], op0=mult, op1=add)

                # deepest-level scan into Y stream 15; guard col for shifts
                nc.vector.memset(Y[:, (M - 1) * S - 1:(M - 1) * S], 0.0)
                nc.vector.tensor_tensor_scan(
                    Y[:, (M - 1) * S:T], pwb(g, 4, S), X4[:, :], 0.0, mult, add)
                Ysh = Y[:, (M - 1) * S - 1:T - 1]   # y^(top) shifted by one

                # E4: -> Y stream 7
                nc.vector.scalar_tensor_tensor(
                    out=Y[:, 7 * S:8 * S], in0=Ysh, scalar=pw(g, 3),
                    in1=X3[:, 0:S], op0=mult, op1=add)
                # E3 edge -> Y stream 3 ; E3 bulk -> Y stream 11
                nc.vector.scalar_tensor_tensor(
                    out=Y[:, 3 * S:4 * S], in0=Ysh, scalar=pw(g, 2),
                    in1=X2[:, 0:S], op0=mult, op1=add)
                nc.vector.scalar_tensor_tensor(
                    out=Y[:, 11 * S:12 * S], in0=Y[:, 7 * S:8 * S], scalar=pw(g, 2),
                    in1=X2[:, 2 * S:3 * S], op0=mult, op1=add)
                # E2 edge -> Y stream 1 (ACT mult + DVE add)
                nc.scalar.mul(TE2e[:, :], Ysh, pw(g, 1))
                nc.vector.tensor_tensor(
                    out=Y[:, S:2 * S], in0=TE2e[:, :], in1=X1[:, 0:S], op=add)
                # E2 bulk -> Y streams {5,9,13} (ACT mult + DVE add)
                nc.scalar.mul(gv(TE2b[:, :], 3, 0, 3, 1),
                              gv(Y[:, :], 16, 3, 12, 4), pw(g, 1))
                nc.vector.tensor_tensor(
                    out=gv(Y[:, :], 16, 5, 14, 4),
                    in0=gv(TE2b[:, :], 3, 0, 3, 1),
                    in1=gv(X1[:, :], 8, 2, 8, 2), op=add)
                # E1 edge -> Y stream 0
                nc.scalar.mul(TE1e[:, :], Ysh, pw(g, 0))
                nc.vector.tensor_tensor(
                    out=Y[:, 0:S], in0=TE1e[:, :], in1=X0[:, 0:S], op=add)
                # E1 bulk -> Y streams {2,4,..,14}
                nc.scalar.mul(gv(TE1b[:, :], 7, 0, 7, 1),
                              gv(Y[:, :], 16, 1, 14, 2), pw(g, 0))
                nc.vector.tensor_tensor(
                    out=gv(Y[:, :], 16, 2, 16, 2),
                    in0=gv(TE1b[:, :], 7, 0, 7, 1),
                    in1=gv(X0[:, :], 16, 2, 16, 2), op=add)

                nc.gpsimd.dma_start(out=y[r0:r0 + 128, :], in_=Y[:, :])

    nc.compile()
    return nc


def _get_nc():
    global _cached
    if _cached is None:
        _cached = _build()
    return _cached


def _perm():
    t = np.arange(T)
    return (t % M) * S + t // M     # device col for time t


def _make_in_maps(data, tau_syn):
    tau = np.asarray(tau_syn, dtype=np.float64)
    beta = np.exp(-DT / tau)  # (1, N) f64
    # beta powers table: [128, NG*NPOW], col g*NPOW+j = beta_chan^(2^j)
    bt = np.empty((128, NG * NPOW), dtype=np.float32)
    for g in range(NG):
        for j in range(NPOW):
            bt[:, g * NPOW + j] = (beta[0, g * 128:(g + 1) * 128] ** (1 << j)
                                   ).astype(np.float32)
    xt = np.asarray(data, dtype=np.float32).transpose(0, 2, 1).astype(np.float16)
    xt = np.ascontiguousarray(xt).reshape(NCORES, ROWS, T)
    # stream-permute all rows except each core's block 0 (rows 0:128)
    perm = _perm()
    xs = xt.copy()
    xs[:, 128:, perm] = xt[:, 128:, :]
    return [{"x": xs[c], "beta": bt} for c in range(NCORES)]


def kernel(data, tau_syn):
    from concourse.bass_utils import run_bass_kernel_spmd

    nc = _get_nc()
    in_maps = _make_in_maps(data, tau_syn)
    res = run_bass_kernel_spmd(nc, in_maps, list(range(NCORES)))
    out = np.stack([res.results[c]["y"] for c in range(NCORES)])  # (8, ROWS, T)
    perm = _perm()
    out[:, 128:, :] = out[:, 128:, perm]
    out = out.astype(np.float32).reshape(B, N, T).transpose(0, 2, 1)
    return np.ascontiguousarray(out)


# revision 7
# speedup vs baseline: 1.2720x; 1.2720x over previous
"""ExpSyn kernel: diagonal linear recurrence isyn_t = beta*isyn_{t-1} + x_t.

The DVE tensor_tensor_scan runs at ~2.12 ns/col regardless of dtype (serial
dependency), so a plain per-row scan is DVE-bound at ~70us/core. This kernel
uses an odd-even (Sklansky-style) decomposition of depth D=4 to convert most
of the scan into elementwise FMAs spread across THREE engines:

  pack level l:  x^(l)_i = beta^(2^(l-1)) * x^(l-1)_{2i} + x^(l-1)_{2i+1}
  scan (T/16 cols) on the deepest level only
  unpack level l: y^(l-1)_{2i} = beta^(2^(l-1)) * y^(l)_{i-1} + x^(l-1)_{2i}
                  y^(l-1)_{2i+1} = y^(l)_i   (free -- already in place)

Data is laid out host-side in a "stream" permutation (col = (t%16)*256 +
t//16) so every pack/unpack is a full-width stride-1 op. The multiply halves
of the big levels run on ACT (0.9 ns/col, per-partition scale), the level-1
add runs on GPSIMD (2.17 ns/col), the rest runs on DVE (tensor_scalar at 4x
= 0.31, tensor_tensor at 2x = 0.55, scalar_tensor_tensor at 1.1 ns/col).
Block 0 keeps the simple chunked plain-scan so the pipeline head stays short
(the tree's ACT->GPS->DVE chain has ~10us latency).

I/O is fp16 (HBM-traffic halving; scan state is fp32 internally; measured
rel err ~5e-4 vs the 2e-2 gate). Loads ride the sync ring, stores the PE
ring, ACT/GPS/DVE do compute only.
"""

import numpy as np

DT = 1e-4
B, T, N = 16, 4096, 512
NCORES = 8
BLOC = B // NCORES          # 2 batches per core
ROWS = BLOC * N             # 1024 scan rows per core
NG = N // 128               # 4 channel groups of 128
NTILES = ROWS // 128        # 8 row-blocks per core
D = 4                       # odd-even tree depth
M = 1 << D                  # 16 streams
S = T >> D                  # 256 cols per stream
NPOW = D + 1                # beta powers 2^0 .. 2^D

_cached = None


def _build():
    import concourse.bacc as bacc
    import concourse.mybir as mybir
    from concourse import tile

    nc = bacc.Bacc("TRN2", debug=False, num_devices=NCORES)
    f32 = mybir.dt.float32
    f16 = mybir.dt.float16
    mult, add = mybir.AluOpType.mult, mybir.AluOpType.add

    x = nc.dram_tensor("x", [ROWS, T], f16, kind="ExternalInput")
    beta_d = nc.dram_tensor("beta", [128, NG * NPOW], f32, kind="ExternalInput")
    y = nc.dram_tensor("y", [ROWS, T], f16, kind="ExternalOutput")

    def gv(ap, ngroups, lo, hi, step):
        """Grouped stream view: [128, ngroups, width] selecting streams."""
        return ap.rearrange("p (g s) -> p g s", g=ngroups)[:, lo:hi:step, :]

    with tile.TileContext(nc) as tc:
        with (
            tc.tile_pool(name="const", bufs=1) as cpool,
            tc.tile_pool(name="work", bufs=4) as wpool,
            tc.tile_pool(name="b0", bufs=1) as b0pool,
        ):
            bsb = cpool.tile([128, NG * NPOW], f32, name="bsb")
            nc.scalar.dma_start(out=bsb[:, :], in_=beta_d[:, :])

            def pw(g, j):            # [128,1] scalar: beta^(2^j) for group g
                return bsb[:, g * NPOW + j:g * NPOW + j + 1]

            def pwb(g, j, n):        # broadcast for the scan
                return pw(g, j).broadcast_to([128, n])

            # ---- block 0: chunked plain scan (short pipeline head) ----
            bounds = [0, 128, 512, 1536, T]
            xt0 = b0pool.tile([128, T], f16, name="xt0")
            for c in range(len(bounds) - 1):
                lo, hi = bounds[c], bounds[c + 1]
                nc.sync.dma_start(out=xt0[:, lo:hi], in_=x[0:128, lo:hi])
            for c in range(len(bounds) - 1):
                lo, hi = bounds[c], bounds[c + 1]
                init = 0.0 if c == 0 else xt0[:, lo - 1:lo]
                nc.vector.tensor_tensor_scan(
                    xt0[:, lo:hi], pwb(0, 0, hi - lo), xt0[:, lo:hi],
                    init, mult, add)
            nc.gpsimd.dma_start(out=y[0:128, :], in_=xt0[:, :])

            # ---- blocks 1..7: odd-even tree, software-pipelined ----
            # front(k): load + ACT L1-mult + GPSIMD L1-add, issued 2 blocks
            # ahead so no engine queue blocks a later block's early stage.
            tiles = {}

            def front(k):
                g = k % NG
                r0 = k * 128
                X0 = wpool.tile([128, T], f16, tag="x0", name=f"x0_{k}")
                TM1 = wpool.tile([128, T // 2], f16, tag="tm1", name=f"tm1_{k}")
                X1 = wpool.tile([128, T // 2], f16, tag="x1", name=f"x1_{k}")
                tiles[k] = (X0, X1)
                nc.sync.dma_start(out=X0[:, :], in_=x[r0:r0 + 128, :])
                nc.scalar.mul(gv(TM1[:, :], 8, 0, 8, 1),
                              gv(X0[:, :], 16, 0, 16, 2), pw(g, 0))
                nc.gpsimd.tensor_tensor(
                    out=X1[:, :].rearrange("p (g s) -> p g s", g=8),
                    in0=gv(TM1[:, :], 8, 0, 8, 1),
                    in1=gv(X0[:, :], 16, 1, 16, 2), op=add)

            front(1)
            front(2)
            for k in range(1, NTILES):
                g = k % NG
                r0 = k * 128
                X0, X1 = tiles.pop(k)
                X2 = wpool.tile([128, T // 4], f16, tag="x2", name=f"x2_{k}")
                X3 = wpool.tile([128, T // 8], f16, tag="x3", name=f"x3_{k}")
                X4 = wpool.tile([128, T // 16], f16, tag="x4", name=f"x4_{k}")
                Y = wpool.tile([128, T], f16, tag="y", name=f"y_{k}")
                TE1b = wpool.tile([128, 7 * S], f16, tag="te1b", name=f"te1b_{k}")
                TE2b = wpool.tile([128, 3 * S], f16, tag="te2b", name=f"te2b_{k}")
                TE1e = wpool.tile([128, S], f16, tag="te1e", name=f"te1e_{k}")
                TE2e = wpool.tile([128, S], f16, tag="te2e", name=f"te2e_{k}")

                # L2..L4 packs: DVE stt
                nc.vector.scalar_tensor_tensor(
                    out=X2[:, :].rearrange("p (g s) -> p g s", g=4),
                    in0=gv(X1[:, :], 8, 0, 8, 2), scalar=pw(g, 1),
                    in1=gv(X1[:, :], 8, 1, 8, 2), op0=mult, op1=add)
                nc.vector.scalar_tensor_tensor(
                    out=X3[:, :].rearrange("p (g s) -> p g s", g=2),
                    in0=gv(X2[:, :], 4, 0, 4, 2), scalar=pw(g, 2),
                    in1=gv(X2[:, :], 4, 1, 4, 2), op0=mult, op1=add)
                nc.vector.scalar_tensor_tensor(
                    out=X4[:, :], in0=X3[:, 0:S], scalar=pw(g, 3),
                    in1=X3[:, S:2 * S], op0=mult, op1=add)

                # deepest-level scan into Y stream 15; guard col for shifts
                nc.vector.memset(Y[:, (M - 1) * S - 1:(M - 1) * S], 0.0)
                nc.vector.tensor_tensor_scan(
                    Y[:, (M - 1) * S:T], pwb(g, 4, S), X4[:, :], 0.0, mult, add)
                Ysh = Y[:, (M - 1) * S - 1:T - 1]   # y^(top) shifted by one

                # E4: -> Y stream 7
                nc.vector.scalar_tensor_tensor(
                    out=Y[:, 7 * S:8 * S], in0=Ysh, scalar=pw(g, 3),
                    in1=X3[:, 0:S], op0=mult, op1=add)
                # E3 edge -> Y stream 3 ; E3 bulk -> Y stream 11
                nc.vector.scalar_tensor_tensor(
                    out=Y[:, 3 * S:4 * S], in0=Ysh, scalar=pw(g, 2),
                    in1=X2[:, 0:S], op0=mult, op1=add)
                nc.vector.scalar_tensor_tensor(
                    out=Y[:, 11 * S:12 * S], in0=Y[:, 7 * S:8 * S], scalar=pw(g, 2),
                    in1=X2[:, 2 * S:3 * S], op0=mult, op1=add)
                # E2 edge -> Y stream 1 (ACT mult + DVE add)
                nc.scalar.mul(TE2e[:, :], Ysh, pw(g, 1))
                nc.vector.tensor_tensor(
                    out=Y[:, S:2 * S], in0=TE2e[:, :], in1=X1[:, 0:S], op=add)
                # E2 bulk -> Y streams {5,9,13} (ACT mult + DVE add)
                nc.scalar.mul(gv(TE2b[:, :], 3, 0, 3, 1),
                              gv(Y[:, :], 16, 3, 12, 4), pw(g, 1))
                nc.vector.tensor_tensor(
                    out=gv(Y[:, :], 16, 5, 14, 4),
                    in0=gv(TE2b[:, :], 3, 0, 3, 1),
                    in1=gv(X1[:, :], 8, 2, 8, 2), op=add)
                # E1 edge -> Y stream 0
                nc.scalar.mul(TE1e[:, :], Ysh, pw(g, 0))
                nc.vector.tensor_tensor(
                    out=Y[:, 0:S], in0=TE1e[:, :], in1=X0[:, 0:S], op=add)
                # E1 bulk -> Y streams {2,4,..,14}
                nc.scalar.mul(gv(TE1b[:, :], 7, 0, 7, 1),
                              gv(Y[:, :], 16, 1, 14, 2), pw(g, 0))
                nc.vector.tensor_tensor(
                    out=gv(Y[:, :], 16, 2, 16, 2),
                    in0=gv(TE1b[:, :], 7, 0, 7, 1),
                    in1=gv(X0[:, :], 16, 2, 16, 2), op=add)

                nc.gpsimd.dma_start(out=y[r0:r0 + 128, :], in_=Y[:, :])
                if k + 2 < NTILES:
                    front(k + 2)

    nc.compile()
    return nc


def _get_nc():
    global _cached
    if _cached is None:
        _cached = _build()
    return _cached


def _perm():
    t = np.arange(T)
    return (t % M) * S + t // M     # device col for time t


def _make_in_maps(data, tau_syn):
    tau = np.asarray(tau_syn, dtype=np.float64)
    beta = np.exp(-DT / tau)  # (1, N) f64
    # beta powers table: [128, NG*NPOW], col g*NPOW+j = beta_chan^(2^j)
    bt = np.empty((128, NG * NPOW), dtype=np.float32)
    for g in range(NG):
        for j in range(NPOW):
            bt[:, g * NPOW + j] = (beta[0, g * 128:(g + 1) * 128] ** (1 << j)
                                   ).astype(np.float32)
    xt = np.asarray(data, dtype=np.float32).transpose(0, 2, 1).astype(np.float16)
    xt = np.ascontiguousarray(xt).reshape(NCORES, ROWS, T)
    # stream-permute all rows except each core's block 0 (rows 0:128)
    perm = _perm()
    xs = xt.copy()
    xs[:, 128:, perm] = xt[:, 128:, :]
    return [{"x": xs[c], "beta": bt} for c in range(NCORES)]


def kernel(data, tau_syn):
    from concourse.bass_utils import run_bass_kernel_spmd

    nc = _get_nc()
    in_maps = _make_in_maps(data, tau_syn)
    res = run_bass_kernel_spmd(nc, in_maps, list(range(NCORES)))
    out = np.stack([res.results[c]["y"] for c in range(NCORES)])  # (8, ROWS, T)
    perm = _perm()
    out[:, 128:, :] = out[:, 128:, perm]
    out = out.astype(np.float32).reshape(B, N, T).transpose(0, 2, 1)
    return np.ascontiguousarray(out)


# revision 8
# speedup vs baseline: 1.3235x; 1.0405x over previous
"""ExpSyn kernel: diagonal linear recurrence isyn_t = beta*isyn_{t-1} + x_t.

The DVE tensor_tensor_scan runs at ~2.12 ns/col regardless of dtype (serial
dependency), so a plain per-row scan is DVE-bound at ~70us/core. This kernel
halves the DVE scan length with a depth-1 odd-even decomposition and spreads
the rest across ACT and GPSIMD:

  pack:   x'_i   = beta * x_{2i} + x_{2i+1}      (ACT mult + GPSIMD add)
  scan:   y_odd  = scan(x', beta^2)              (DVE, T/2 cols)
  unpack: y_{2i} = beta * y_odd_{i-1} + x_{2i}   (ACT mult + DVE add)

Host lays x out de-interleaved (evens in cols 0:T/2, odds in T/2:T) so every
op is a full-width stride-1 2D AP. Software-pipelined: the load+pack front
runs 2 blocks ahead so no engine queue blocks a later block's early stage.
Block 0 keeps the simple chunked plain-scan so the pipeline head stays short.

I/O is fp16 (halves HBM traffic; the scan carries fp32 state internally;
rel err ~1e-3 vs the 2e-2 gate). Loads ride the sync ring, stores the
GPSIMD ring.
"""

import numpy as np

DT = 1e-4
B, T, N = 16, 4096, 512
NCORES = 8
BLOC = B // NCORES          # 2 batches per core
ROWS = BLOC * N             # 1024 scan rows per core
NG = N // 128               # 4 channel groups of 128
NTILES = ROWS // 128        # 8 row-blocks per core
H = T // 2                  # 2048
NPOW = 2                    # beta, beta^2

_cached = None


def _build():
    import concourse.bacc as bacc
    import concourse.mybir as mybir
    from concourse import tile

    nc = bacc.Bacc("TRN2", debug=False, num_devices=NCORES)
    f32 = mybir.dt.float32
    f16 = mybir.dt.float16
    mult, add = mybir.AluOpType.mult, mybir.AluOpType.add

    x = nc.dram_tensor("x", [ROWS, T], f16, kind="ExternalInput")
    beta_d = nc.dram_tensor("beta", [128, NG * NPOW], f32, kind="ExternalInput")
    y = nc.dram_tensor("y", [ROWS, T], f16, kind="ExternalOutput")

    with tile.TileContext(nc) as tc:
        with (
            tc.tile_pool(name="const", bufs=1) as cpool,
            tc.tile_pool(name="work", bufs=4) as wpool,
            tc.tile_pool(name="b0", bufs=1) as b0pool,
        ):
            bsb = cpool.tile([128, NG * NPOW], f32, name="bsb")
            nc.scalar.dma_start(out=bsb[:, :], in_=beta_d[:, :])

            def pw(g, j):            # [128,1] scalar: beta^(2^j) for group g
                return bsb[:, g * NPOW + j:g * NPOW + j + 1]

            def pwb(g, j, n):        # broadcast for the scan
                return pw(g, j).broadcast_to([128, n])

            # ---- block 0: chunked plain scan (short pipeline head) ----
            bounds = [0, 128, 512, 1536, T]
            xt0 = b0pool.tile([128, T], f16, name="xt0")
            for c in range(len(bounds) - 1):
                lo, hi = bounds[c], bounds[c + 1]
                nc.sync.dma_start(out=xt0[:, lo:hi], in_=x[0:128, lo:hi])
            for c in range(len(bounds) - 1):
                lo, hi = bounds[c], bounds[c + 1]
                init = 0.0 if c == 0 else xt0[:, lo - 1:lo]
                nc.vector.tensor_tensor_scan(
                    xt0[:, lo:hi], pwb(0, 0, hi - lo), xt0[:, lo:hi],
                    init, mult, add)
            nc.gpsimd.dma_start(out=y[0:128, :], in_=xt0[:, :])

            # ---- blocks 1..7: depth-1 odd-even, software-pipelined ----
            tiles = {}

            def front(k):
                g = k % NG
                r0 = k * 128
                X0 = wpool.tile([128, T], f16, tag="x0", name=f"x0_{k}")
                TM1 = wpool.tile([128, H], f16, tag="tm1", name=f"tm1_{k}")
                X1 = wpool.tile([128, H], f16, tag="x1", name=f"x1_{k}")
                tiles[k] = (X0, X1)
                nc.sync.dma_start(out=X0[:, :], in_=x[r0:r0 + 128, :])
                # pack: TM1 = beta * x_even ; X1 = TM1 + x_odd
                nc.scalar.mul(TM1[:, :], X0[:, 0:H], pw(g, 0))
                nc.gpsimd.tensor_tensor(
                    out=X1[:, :], in0=TM1[:, :], in1=X0[:, H:T], op=add)

            front(1)
            front(2)
            for k in range(1, NTILES):
                g = k % NG
                r0 = k * 128
                X0, X1 = tiles.pop(k)
                Y = wpool.tile([128, T], f16, tag="y", name=f"y_{k}")
                TE = wpool.tile([128, H], f16, tag="te", name=f"te_{k}")

                # odd outputs: scan of the packed stream -> Y[:, H:T]
                nc.vector.memset(Y[:, H - 1:H], 0.0)
                nc.vector.tensor_tensor_scan(
                    Y[:, H:T], pwb(g, 1, H), X1[:, :], 0.0, mult, add)
                # even outputs: beta * y_odd_{i-1} + x_even
                nc.scalar.mul(TE[:, :], Y[:, H - 1:T - 1], pw(g, 0))
                nc.vector.tensor_tensor(
                    out=Y[:, 0:H], in0=TE[:, :], in1=X0[:, 0:H], op=add)

                nc.gpsimd.dma_start(out=y[r0:r0 + 128, :], in_=Y[:, :])
                if k + 2 < NTILES:
                    front(k + 2)

    nc.compile()
    return nc


def _get_nc():
    global _cached
    if _cached is None:
        _cached = _build()
    return _cached


def _perm():
    t = np.arange(T)
    return (t % 2) * H + t // 2     # device col for time t


def _make_in_maps(data, tau_syn):
    tau = np.asarray(tau_syn, dtype=np.float64)
    beta = np.exp(-DT / tau)  # (1, N) f64
    bt = np.empty((128, NG * NPOW), dtype=np.float32)
    for g in range(NG):
        for j in range(NPOW):
            bt[:, g * NPOW + j] = (beta[0, g * 128:(g + 1) * 128] ** (1 << j)
                                   ).astype(np.float32)
    xt = np.asarray(data, dtype=np.float32).transpose(0, 2, 1).astype(np.float16)
    xt = np.ascontiguousarray(xt).reshape(NCORES, ROWS, T)
    perm = _perm()
    xs = xt.copy()
    xs[:, 128:, perm] = xt[:, 128:, :]
    return [{"x": xs[c], "beta": bt} for c in range(NCORES)]


def kernel(data, tau_syn):
    from concourse.bass_utils import run_bass_kernel_spmd

    nc = _get_nc()
    in_maps = _make_in_maps(data, tau_syn)
    res = run_bass_kernel_spmd(nc, in_maps, list(range(NCORES)))
    out = np.stack([res.results[c]["y"] for c in range(NCORES)])  # (8, ROWS, T)
    perm = _perm()
    out[:, 128:, :] = out[:, 128:, perm]
    out = out.astype(np.float32).reshape(B, N, T).transpose(0, 2, 1)
    return np.ascontiguousarray(out)


# revision 9
# speedup vs baseline: 1.5220x; 1.1500x over previous
"""ExpSyn kernel: diagonal linear recurrence isyn_t = beta*isyn_{t-1} + x_t.

Strategy:
  - Host: transpose data (B,T,N) -> (B,N,T) so time is contiguous per channel,
    and downcast to fp16 (tensor_tensor_scan carries its state in fp32
    regardless of operand dtype, so only the I/O is rounded; measured rel err
    ~5e-4 vs the 2e-2 gate). Halves HBM traffic vs fp32 -> ~2x on this
    memory-bound kernel.
  - Shard batch over 8 cores (2 batches/core -> 1024 rows of length T=4096).
  - Device: per 128-row block, 1MB fp16 DMA load (nc.sync ring), DVE
    tensor_tensor_scan (state = beta*state + x along the free/time dim,
    fp32 state, fp16 in/out), 1MB fp16 DMA store (nc.scalar ring — separate
    HWDGE FIFO so stores never head-of-line block loads). First block loads
    in geometric chunks (chained via initial=) so the DVE starts early;
    middle blocks scan/store in halves to smooth store bandwidth; last block
    stores in chunks so the tail is short.
  - Host: upcast fp16 -> fp32, gather, transpose back to (B,T,N).
"""

import numpy as np

DT = 1e-4
B, T, N = 16, 4096, 512
NCORES = 8
BLOC = B // NCORES          # 2 batches per core
ROWS = BLOC * N             # 1024 scan rows per core
NG = N // 128               # 4 channel groups of 128
NTILES = ROWS // 128        # 8 row-blocks per core

_cached = None


def _build():
    """Build + compile the single-core Bass program (run SPMD on 8 cores)."""
    import concourse.bacc as bacc
    import concourse.mybir as mybir
    from concourse import tile

    nc = bacc.Bacc("TRN2", debug=False, num_devices=NCORES)
    f32 = mybir.dt.float32
    f16 = mybir.dt.float16
    mult, add = mybir.AluOpType.mult, mybir.AluOpType.add

    x = nc.dram_tensor("x", [ROWS, T], f16, kind="ExternalInput")
    beta_d = nc.dram_tensor("beta", [128, NG], f32, kind="ExternalInput")
    y = nc.dram_tensor("y", [ROWS, T], f16, kind="ExternalOutput")

    with tile.TileContext(nc) as tc:
        with (
            tc.tile_pool(name="const", bufs=1) as cpool,
            tc.tile_pool(name="xin", bufs=8) as xpool,
        ):
            # tiny beta DMA rides the ACT ring (idle until the first store,
            # so it lands well before the first scan needs it)
            bsb = cpool.tile([128, NG], f32, name="bsb")
            nc.scalar.dma_start(out=bsb[:, :], in_=beta_d[:, :])

            def bcast(g, n):
                return bsb[:, g:g + 1].broadcast_to([128, n])

            # ---- block 0: chunked loads so the DVE starts ASAP ----
            # geometric chunk sizes: tiny first chunk -> earliest scan start
            bounds = [0, 128, 512, 1536, T]
            xt0 = xpool.tile([128, T], f16, tag="xt", name="xt0")
            for c in range(len(bounds) - 1):
                lo, hi = bounds[c], bounds[c + 1]
                nc.sync.dma_start(out=xt0[:, lo:hi], in_=x[0:128, lo:hi])
            for c in range(len(bounds) - 1):
                lo, hi = bounds[c], bounds[c + 1]
                init = 0.0 if c == 0 else xt0[:, lo - 1:lo]
                nc.vector.tensor_tensor_scan(
                    xt0[:, lo:hi], bcast(0, hi - lo), xt0[:, lo:hi],
                    init, mult, add)
            nc.scalar.dma_start(out=y[0:128, :], in_=xt0[:, :])

            # ---- blocks 1..6: 1MB load; scan + store in halves so the
            # store stream starts mid-scan and bandwidth stays smooth ----
            H = T // 2
            for k in range(1, NTILES - 1):
                g = k % NG
                xt = xpool.tile([128, T], f16, tag="xt", name=f"xt{k}")
                nc.sync.dma_start(out=xt[:, :], in_=x[k * 128:(k + 1) * 128, :])
                nc.vector.tensor_tensor_scan(
                    xt[:, 0:H], bcast(g, H), xt[:, 0:H], 0.0, mult, add)
                nc.scalar.dma_start(out=y[k * 128:(k + 1) * 128, 0:H],
                                    in_=xt[:, 0:H])
                nc.vector.tensor_tensor_scan(
                    xt[:, H:T], bcast(g, H), xt[:, H:T],
                    xt[:, H - 1:H], mult, add)
                nc.scalar.dma_start(out=y[k * 128:(k + 1) * 128, H:T],
                                    in_=xt[:, H:T])

            # ---- block 7: chunked stores so the tail is short ----
            k = NTILES - 1
            xt7 = xpool.tile([128, T], f16, tag="xt", name="xt7")
            nc.sync.dma_start(out=xt7[:, :], in_=x[k * 128:(k + 1) * 128, :])
            g = k % NG
            # shrinking chunks so the very last store is only 128KB
            bounds7 = [0, 1536, 2560, 3584, T]
            for c in range(len(bounds7) - 1):
                lo, hi = bounds7[c], bounds7[c + 1]
                init = 0.0 if c == 0 else xt7[:, lo - 1:lo]
                nc.vector.tensor_tensor_scan(
                    xt7[:, lo:hi], bcast(g, hi - lo), xt7[:, lo:hi],
                    init, mult, add)
                nc.scalar.dma_start(out=y[k * 128:(k + 1) * 128, lo:hi],
                                    in_=xt7[:, lo:hi])

    nc.compile()
    return nc


def _get_nc():
    global _cached
    if _cached is None:
        _cached = _build()
    return _cached


def _make_in_maps(data, tau_syn):
    tau = np.asarray(tau_syn, dtype=np.float64)
    beta = np.exp(-DT / tau).astype(np.float32)  # (1, N)
    beta_g = np.ascontiguousarray(beta.reshape(NG, 128).T)  # (128, NG)
    # (B, T, N) -> (B, N, T), batch-sharded across cores, fp16
    xt = np.asarray(data, dtype=np.float32).transpose(0, 2, 1).astype(np.float16)
    xt = np.ascontiguousarray(xt).reshape(NCORES, ROWS, T)
    return [{"x": xt[c], "beta": beta_g} for c in range(NCORES)]


def kernel(data, tau_syn):
    from concourse.bass_utils import run_bass_kernel_spmd

    nc = _get_nc()
    in_maps = _make_in_maps(data, tau_syn)
    res = run_bass_kernel_spmd(nc, in_maps, list(range(NCORES)))
    out = np.stack([res.results[c]["y"] for c in range(NCORES)])  # (8, ROWS, T)
    out = out.astype(np.float32).reshape(B, N, T).transpose(0, 2, 1)  # (B, T, N)
    return np.ascontiguousarray(out)
